# revision 1
# baseline (speedup 1.0000x reference)
"""Multi-head attention kernel for 8 Trainium2 NeuronCores.

Problem: x[4, 2048, 1024], 16 heads x 64 head-dim MHA (QKV proj -> softmax
attention -> out proj), fp32.

Sharding: 8 cores = 4 batches x 2 head-groups. Core c handles batch c//2 and
heads (c%2)*8 .. (c%2)*8+7 (tensor-parallel split of the QKV/out projections).
Each core computes a partial output [2048, 1024] (its 8 heads through Wo);
the host sums the two partials per batch and adds bo.

Per-core kernel (all matmuls in fp32r = full-rate TF32-like):
  A. transpose x via PE -> xT [dim, seq]
  B. QKV: Q^T,K^T [hd, seq] (K^T scaled by 1/8), V in natural [seq, hd] layout
     augmented with a ones column per head (Vaug) for the softmax denominator
  C. per head-pair, per 512-wide q-chunk, loop over 16 k-tiles:
     scores S^T[k,q] = K^T.T @ Q^T for both heads (row-packed, K=64 each),
     exp on ScalarE (no max subtraction needed; |scores| ~ few units),
     U^T[hd+1, q] += Vaug.T @ exp(S^T)  (row 64 = softmax denominator r),
     then C^T = U^T * (1/r) broadcast via DMA, stored to UT tiles
  D. out = C @ Wo per seq-tile (contraction over all 512 local hd dims)
"""

import numpy as np

B = 4
SEQ = 2048
DIM = 1024
NH_LOC = 8      # heads per core
HID = 64
HDL = NH_LOC * HID  # 512
N_CORES = 8

_PROG = None


def _build_program(seq=SEQ, reps=1):
    import contextlib

    import concourse.bass as bass
    import concourse.mybir as mybir
    import concourse.tile as tile
    from concourse import bacc
    from concourse.masks import make_identity

    FP32 = mybir.dt.float32
    FP32R = mybir.dt.float32r
    Exp = mybir.ActivationFunctionType.Exp
    Alu = mybir.AluOpType

    seq_t = seq // 128            # seq tiles
    dim_t = DIM // 128            # 8
    n_half = 2 if seq >= 1024 else 1
    sh_seq = seq // n_half        # seq cols per half
    sh_t = seq_t // n_half        # seq tiles per half
    sh_c = sh_seq // 512          # 512-chunks per half
    n_qc = seq // 512             # q chunks
    n_hp = NH_LOC // 2            # head pairs = 4
    n_m = HDL // 128              # hd-row tiles = 4

    nc = bacc.Bacc()
    x_d = nc.declare_dram_parameter("x", [seq, DIM], FP32, isOutput=False)
    wq_d = nc.declare_dram_parameter("wq", [DIM, HDL], FP32, isOutput=False)
    wk_d = nc.declare_dram_parameter("wk", [DIM, HDL], FP32, isOutput=False)
    wv_d = nc.declare_dram_parameter("wv", [DIM, HDL], FP32, isOutput=False)
    bq_d = nc.declare_dram_parameter("bq", [HDL], FP32, isOutput=False)
    bk_d = nc.declare_dram_parameter("bk", [HDL], FP32, isOutput=False)
    bv_d = nc.declare_dram_parameter("bv", [HDL], FP32, isOutput=False)
    wo_d = nc.declare_dram_parameter("wo", [HDL, DIM], FP32, isOutput=False)
    out_d = nc.declare_dram_parameter("out", [seq, DIM], FP32, isOutput=True)
    rrs_d = nc.dram_tensor("rrs", [NH_LOC // 2, seq // 512, 2, 512], FP32)

    with tile.TileContext(nc, pool_alloc_mode="queue") as tc:
        with (
            tc.tile_pool(name="persist", bufs=1) as persist,
        ):
            ident = persist.tile([128, 128], FP32)
            make_identity(nc, ident[:])

            QT = [[persist.tile([128, sh_seq], FP32R, tag=f"qt{m}_{h}", name=f"qt{m}_{h}")
                   for h in range(n_half)] for m in range(n_m)]
            KT = [[persist.tile([128, sh_seq], FP32R, tag=f"kt{m}_{h}", name=f"kt{m}_{h}")
                   for h in range(n_half)] for m in range(n_m)]
            Vaug = [persist.tile([128, NH_LOC * (HID + 1)], FP32R, tag=f"va{st}", name=f"va{st}")
                    for st in range(seq_t)]

            bq_sb = persist.tile([128, n_m], FP32)
            bk_sb = persist.tile([128, n_m], FP32)
            nc.sync.dma_start(out=bq_sb[:], in_=bq_d[:].rearrange("(m p) -> p m", p=128))
            nc.sync.dma_start(out=bk_sb[:], in_=bk_d[:].rearrange("(m p) -> p m", p=128))
            bv_bc = persist.tile([128, HDL], FP32)
            bv_ap = bv_d[:]
            nc.sync.dma_start(
                out=bv_bc[:],
                in_=bass.AP(tensor=bv_ap.tensor, offset=bv_ap.offset,
                            ap=[[0, 128], [1, HDL]]),
            )
            ones8 = persist.tile([128, NH_LOC], FP32)
            nc.vector.memset(ones8[:], 1.0)

            rep_ctx = tc.For_i(0, reps, 1) if reps > 1 else contextlib.nullcontext()
            with rep_ctx:
                _build_body(nc, tc, bass, mybir, tile, make_identity, locals())

    nc.compile()
    return nc


def _build_body(nc, tc, bass, mybir, tile, make_identity, env):
    FP32 = mybir.dt.float32
    FP32R = mybir.dt.float32r
    Exp = mybir.ActivationFunctionType.Exp
    Alu = mybir.AluOpType
    seq = env["seq"]
    seq_t, dim_t, n_half = env["seq_t"], env["dim_t"], env["n_half"]
    sh_seq, sh_t, sh_c = env["sh_seq"], env["sh_t"], env["sh_c"]
    n_qc, n_hp, n_m = env["n_qc"], env["n_hp"], env["n_m"]
    persist = env["persist"]
    ident, QT, KT, Vaug = env["ident"], env["QT"], env["KT"], env["Vaug"]
    bq_sb, bk_sb, bv_bc, ones8 = env["bq_sb"], env["bk_sb"], env["bv_bc"], env["ones8"]
    x_d, wq_d, wk_d, wv_d = env["x_d"], env["wq_d"], env["wk_d"], env["wv_d"]
    wo_d, out_d, rrs_d = env["wo_d"], env["out_d"], env["rrs_d"]

    if True:
            # ---------- Phase A+B: transpose x, QKV projections ----------
            with (
                tc.tile_pool(name="xstage", bufs=3) as xstage,
                tc.tile_pool(name="xtp", bufs=1) as xtp,
                tc.tile_pool(name="wstage", bufs=3) as wstage,
                tc.tile_pool(name="wpool", bufs=2) as wpool,
                tc.tile_pool(name="tpp", bufs=2, space="PSUM") as tpp,
                tc.tile_pool(name="qkvp", bufs=4, space="PSUM") as qkvp,
            ):
                for sh in range(n_half):
                    # xTall column layout: [dim-tile d][seq col] (d-major)
                    xTall = xtp.tile([128, dim_t * sh_seq], FP32R, tag="xtall",
                                     name="xtall")
                    xT = [xTall[:, d*sh_seq:(d+1)*sh_seq] for d in range(dim_t)]
                    for st8 in range(sh_t):
                        st = sh * sh_t + st8
                        xst = xstage.tile([128, DIM], FP32, tag="xst")
                        dma_eng = nc.sync if st % 2 == 0 else nc.scalar
                        dma_eng.dma_start(out=xst[:], in_=x_d[st*128:(st+1)*128, :])
                        for dg in range(dim_t // 4):
                            tp = tpp.tile([128, 512], FP32, tag="tp")
                            for j in range(4):
                                d = dg * 4 + j
                                nc.tensor.transpose(
                                    tp[:, j*128:(j+1)*128],
                                    xst[:, d*128:(d+1)*128], ident[:])
                            # one strided copy: 4 transposed blocks -> 4 xT tiles
                            out_ap = xTall[:].rearrange(
                                "p (d s) -> p d s", d=dim_t)[
                                :, dg*4:(dg+1)*4, st8*128:(st8+1)*128]
                            nc.vector.tensor_copy(
                                out_ap,
                                tp[:].rearrange("p (j c) -> p j c", c=128))

                    for proj, w_dram, dst, bias, scale in (
                        ("k", wk_d, KT, bk_sb, 0.125),
                        ("v", wv_d, None, None, None),
                        ("q", wq_d, QT, bq_sb, None),
                    ):
                        wr = []
                        for d in range(dim_t):
                            wst = wstage.tile([128, HDL], FP32, tag="wst")
                            nc.sync.dma_start(out=wst[:], in_=w_dram[d*128:(d+1)*128, :])
                            wrd = wpool.tile([128, HDL], FP32R, tag=f"w{d}")
                            nc.vector.tensor_copy(wrd[:], wst[:])
                            wr.append(wrd)
                        if proj != "v":
                            # dst[m][:, cols] = (x @ W + b)^T (scaled for K)
                            for m in range(n_m):
                                for sc in range(sh_c):
                                    qp = qkvp.tile([128, 512], FP32, tag="qp")
                                    for d in range(dim_t):
                                        nc.tensor.matmul(
                                            qp[:],
                                            wr[d][:, m*128:(m+1)*128],
                                            xT[d][:, sc*512:(sc+1)*512],
                                            start=(d == 0), stop=(d == dim_t - 1),
                                        )
                                    col0 = sc * 512
                                    if scale is None:
                                        nc.vector.tensor_scalar(
                                            dst[m][sh][:, col0:col0+512], qp[:],
                                            bias[:, m:m+1], None, Alu.add)
                                    else:
                                        nc.vector.tensor_scalar(
                                            dst[m][sh][:, col0:col0+512], qp[:],
                                            bias[:, m:m+1], scale, Alu.add, Alu.mult)
                        else:
                            # V natural [seq, hd] + bias, strided into Vaug
                            for st8 in range(sh_t):
                                st = sh * sh_t + st8
                                vp = qkvp.tile([128, HDL], FP32, tag="qp")
                                for d in range(dim_t):
                                    nc.tensor.matmul(
                                        vp[:],
                                        xT[d][:, st8*128:(st8+1)*128],
                                        wr[d][:],
                                        start=(d == 0), stop=(d == dim_t - 1),
                                    )
                                va3 = Vaug[st][:].rearrange("p (h c) -> p h c", c=HID+1)
                                nc.vector.tensor_tensor(
                                    va3[:, :, 0:HID],
                                    vp[:].rearrange("p (h c) -> p h c", c=HID),
                                    bv_bc[:].rearrange("p (h c) -> p h c", c=HID),
                                    Alu.add)
                                nc.vector.tensor_copy(
                                    va3[:, :, HID:HID+1],
                                    ones8[:].rearrange("p (h c) -> p h c", c=1))

            # ---------- Phase C+D: attention + output projection ----------
            # Two independent (hp, qc) streams are interleaved per kt step so
            # ScalarE (exp, the critical resource) never waits on the PE
            # scores->attnV chain of a single stream. The output projection
            # for a q-chunk pair runs as soon as all head-pairs finished it,
            # hiding phase D under the next chunk's attention.
            utpool_cm = tc.tile_pool(name="utpool", bufs=1)
            utpool = utpool_cm.__enter__()
            UT = [[utpool.tile([128, 512], FP32R, tag=f"ut{hp}_{q}", name=f"ut{hp}_{q}")
                   for q in range(n_qc)] for hp in range(n_hp)]
            with (
                tc.tile_pool(name="epool", bufs=6) as epool,
                tc.tile_pool(name="rpool", bufs=4) as rpool,
                tc.tile_pool(name="rbpool", bufs=6) as rbpool,
                tc.tile_pool(name="sps", bufs=2, space="PSUM") as sps,
                tc.tile_pool(name="ups", bufs=1, space="PSUM") as ups,
            ):
                def attn_stream(sid, hp, qc):
                    """Emit one (hp, qc) attention unit using stream slot sid."""
                    vca = 2 * hp * (HID + 1)
                    vcb = (2 * hp + 1) * (HID + 1)
                    ua = ups.tile([HID + 1, 512], FP32, tag=f"ua{sid}",
                                  name=f"ua{sid}")
                    ub = ups.tile([HID + 1, 512], FP32, tag=f"ub{sid}",
                                  name=f"ub{sid}")
                    # scores+exp emitted at kt; the dependent attnV matmuls
                    # are emitted one kt later so the in-order PE queue never
                    # head-of-line blocks waiting for the exp.
                    steps = []
                    attns = []
                    for kt in range(seq_t):
                        def step(kt=kt, hp=hp, qc=qc, vca=vca, vcb=vcb,
                                 ua=ua, ub=ub):
                            s2 = sps.tile([128, 1024], FP32, tag="s2", name="s2")
                            kth = KT[hp][kt // sh_t]
                            kc0 = (kt % sh_t) * 128
                            qth = QT[hp][(qc * 512) // sh_seq]
                            qc0 = (qc * 512) % sh_seq
                            nc.tensor.matmul(
                                s2[:, 0:512],
                                kth[0:64, kc0:kc0+128],
                                qth[0:64, qc0:qc0+512],
                                start=True, stop=True)
                            nc.tensor.matmul(
                                s2[:, 512:1024],
                                kth[64:128, kc0:kc0+128],
                                qth[64:128, qc0:qc0+512],
                                start=True, stop=True)
                            e2 = epool.tile([128, 1024], FP32R, tag="e2", name="e2")
                            nc.scalar.activation(e2[:], s2[:], Exp)
                            def attn(e2=e2, kt=kt, ua=ua, ub=ub,
                                     vca=vca, vcb=vcb):
                                nc.tensor.matmul(
                                    ua[:], Vaug[kt][:, vca:vca+HID+1],
                                    e2[:, 0:512],
                                    start=(kt == 0), stop=(kt == seq_t - 1))
                                nc.tensor.matmul(
                                    ub[:], Vaug[kt][:, vcb:vcb+HID+1],
                                    e2[:, 512:1024],
                                    start=(kt == 0), stop=(kt == seq_t - 1))
                            attns.append(attn)
                        steps.append(step)

                    def finish(ua=ua, ub=ub, hp=hp, qc=qc):
                        for hi, (ui, rowbase) in enumerate(((ua, 0), (ub, 64))):
                            # copy U psum->sbuf first so the psum bank frees
                            # quickly; normalize from the sbuf copy.
                            usb = rbpool.tile([HID + 1, 512], FP32, tag="usb")
                            nc.vector.tensor_copy(usb[:], ui[:])
                            rr = rpool.tile([1, 512], FP32, tag="rr")
                            nc.vector.reciprocal(rr[:], usb[HID:HID+1, :])
                            slot = rrs_d[hp, qc, hi, :]
                            nc.sync.dma_start(out=slot, in_=rr[0:1, :])
                            rb = rbpool.tile([HID, 512], FP32, tag="rb")
                            nc.sync.dma_start(
                                out=rb[:],
                                in_=bass.AP(tensor=slot.tensor,
                                            offset=slot.offset,
                                            ap=[[0, HID], [1, 512]]))
                            nc.vector.tensor_tensor(
                                UT[hp][qc][rowbase:rowbase+HID, :],
                                usb[0:HID, :], rb[:], Alu.mult)
                    return steps, attns, finish

                # (hp, qc) units in qc-major order; run two streams at a time.
                units = [(hp, qc) for qc in range(n_qc) for hp in range(n_hp)]
                for u in range(0, len(units), 2):
                    pair = units[u:u+2]
                    streams = [attn_stream(i, hp, qc)
                               for i, (hp, qc) in enumerate(pair)]
                    for kt in range(seq_t):
                        for stream in streams:
                            stream[0][kt]()      # scores + exp
                        if kt > 0:
                            for stream in streams:
                                stream[1][kt - 1]()  # attnV of previous kt
                    for stream in streams:
                        stream[1][seq_t - 1]()
                    for stream in streams:
                        stream[2]()

            # ---------- Phase D: output projection ----------
            with (
                tc.tile_pool(name="wostage", bufs=2) as wostage,
                tc.tile_pool(name="wopool", bufs=1) as wopool,
                tc.tile_pool(name="outstage", bufs=3) as outstage,
                tc.tile_pool(name="ops", bufs=4, space="PSUM") as ops,
            ):
                wo_r = []
                for hp in range(n_hp):
                    wos = wostage.tile([128, DIM], FP32, tag="wos")
                    nc.sync.dma_start(out=wos[:], in_=wo_d[hp*128:(hp+1)*128, :])
                    wr = wopool.tile([128, DIM], FP32R, tag=f"wo{hp}", name=f"wo{hp}")
                    nc.vector.tensor_copy(wr[:], wos[:])
                    wo_r.append(wr)
                for st in range(seq_t):
                    ot = outstage.tile([128, DIM], FP32, tag="ot")
                    for oc in range(DIM // 512):
                        op_t = ops.tile([128, 512], FP32, tag="op")
                        for hp in range(n_hp):
                            nc.tensor.matmul(
                                op_t[:],
                                UT[hp][st // 4][:, (st % 4)*128:(st % 4)*128+128],
                                wo_r[hp][:, oc*512:(oc+1)*512],
                                start=(hp == 0), stop=(hp == n_hp - 1))
                        # split psum->sbuf copies between DVE and idle ScalarE
                        if oc == 0:
                            nc.vector.tensor_copy(ot[:, oc*512:(oc+1)*512],
                                                  op_t[:])
                        else:
                            nc.scalar.copy(ot[:, oc*512:(oc+1)*512], op_t[:])
                    (nc.sync if st % 2 == 0 else nc.scalar).dma_start(
                        out=out_d[st*128:(st+1)*128, :], in_=ot[:])
            utpool_cm.__exit__(None, None, None)


def _get_program():
    global _PROG
    if _PROG is None:
        _PROG = _build_program()
    return _PROG


def _make_in_maps(inputs):
    x = np.asarray(inputs["x"], dtype=np.float32)
    in_maps = []
    for c in range(N_CORES):
        b, g = divmod(c, 2)
        sl = slice(g * HDL, (g + 1) * HDL)
        in_maps.append({
            "x": np.ascontiguousarray(x[b]),
            "wq": np.ascontiguousarray(np.asarray(inputs["Wq"], np.float32)[:, sl]),
            "wk": np.ascontiguousarray(np.asarray(inputs["Wk"], np.float32)[:, sl]),
            "wv": np.ascontiguousarray(np.asarray(inputs["Wv"], np.float32)[:, sl]),
            "bq": np.ascontiguousarray(np.asarray(inputs["bq"], np.float32)[sl]),
            "bk": np.ascontiguousarray(np.asarray(inputs["bk"], np.float32)[sl]),
            "bv": np.ascontiguousarray(np.asarray(inputs["bv"], np.float32)[sl]),
            "wo": np.ascontiguousarray(np.asarray(inputs["Wo"], np.float32)[sl, :]),
        })
    return in_maps


def kernel(x, Wq, bq, Wk, bk, Wv, bv, Wo, bo):
    from concourse.bass_utils import run_bass_kernel_spmd

    bo = np.asarray(bo, dtype=np.float32)
    nc = _get_program()
    in_maps = _make_in_maps(dict(x=x, Wq=Wq, bq=bq, Wk=Wk, bk=bk, Wv=Wv, bv=bv,
                                 Wo=Wo, bo=bo))
    res = run_bass_kernel_spmd(nc, in_maps, core_ids=list(range(N_CORES)))
    out = np.empty((B, SEQ, DIM), dtype=np.float32)
    for b in range(B):
        out[b] = res.results[2 * b]["out"] + res.results[2 * b + 1]["out"] + bo
    return out



# revision 7
# speedup vs baseline: 1.4989x; 1.4989x over previous
"""Multi-head attention kernel for 8 Trainium2 NeuronCores.

Problem: x[4, 2048, 1024], 16 heads x 64 head-dim MHA (QKV proj -> softmax
attention -> out proj), fp32 reference, rel-err gate 2e-2.

Sharding: 8 cores = 4 batches x 2 head-groups. Core c handles batch c//2 and
heads (c%2)*8 .. (c%2)*8+7. Each core computes a partial output [2048, 1024]
(its 8 heads through Wo); the host sums the two partials per batch and adds
bo + bv @ Wo (bias folds: bk drops out of softmax entirely; bv commutes
through the attention average into a constant output offset).

Precision design (validated in numpy sim, rel ~1.4e-2):
  - QKV projections: bf16 inputs (x^T and W pre-cast on host), fp32 psum.
  - Scores: fp8 DoubleRow matmul. Q is stored scaled by 8 with bias folded
    in as an fp8 (value, residual) pair; K is stored fp8 duplicated across
    the two DR k-slabs, so one DR matmul computes K8.(Q8a+Q8b) = K8.Qexact.
    Only the K-side fp8 error remains.
  - exp: stream A (even head-pairs) exact Exp on ScalarE -> fp8; stream B
    Schraudolph-style bit-trick on DVE: i8 = s*c1 + 56.5 cast to int8 IS the
    e4m3 bit pattern of ~exp(s/64). The constant-offset error cancels in the
    softmax normalization.
  - attn.V: two fp8 DoubleRow matmuls per key-pair against (V8, V-residual)
    tiles, killing the V quantization error. A 1/32 ones-column in the V8
    tile accumulates the softmax denominator in psum row 64.
  - ctx and out-projection: bf16 (fp8 ctx would cost 1.8e-2 alone).

Per-core engine usage: PE ~426k cycles of matmuls; exp work is split between
ScalarE (exact) and DVE (bit-trick); DVE also does residuals + normalize.
"""

import numpy as np
import ml_dtypes

B = 4
SEQ = 2048
DIM = 1024
NH_LOC = 8      # heads per core
HID = 64
HDL = NH_LOC * HID  # 512
N_CORES = 8

BF16 = ml_dtypes.bfloat16
E4M3 = ml_dtypes.float8_e4m3

# exp(s/64) ~= bits(s * SCHRAUD_MUL + SCHRAUD_OFF) as e4m3
SCHRAUD_MUL = float(8.0 * np.log2(np.e) / 64.0)
SCHRAUD_OFF = 56.5
EXP_SCALE = 1.0 / 64.0

_PROG = None


def _build_program(seq=SEQ, reps=1):
    import contextlib

    import concourse.bass as bass
    import concourse.mybir as mybir
    import concourse.tile as tile
    from concourse import bacc

    FP32 = mybir.dt.float32
    BF = mybir.dt.bfloat16
    F8 = mybir.dt.float8e4
    I8 = mybir.dt.int8
    Exp = mybir.ActivationFunctionType.Exp
    Ident = mybir.ActivationFunctionType.Identity
    Copy = mybir.ActivationFunctionType.Copy
    Alu = mybir.AluOpType
    DR = mybir.MatmulPerfMode.DoubleRow

    seq_t = seq // 128            # 16 key tiles
    n_qc = seq // 512             # 4 q chunks
    n_ktp = seq_t // 2            # 8 key-tile pairs
    dim_t = DIM // 128            # 8

    nc = bacc.Bacc()
    xt_d = nc.declare_dram_parameter("xt", [128, dim_t * seq], BF, isOutput=False)
    wq_d = nc.declare_dram_parameter("wq", [128, dim_t * HDL], BF, isOutput=False)
    wk_d = nc.declare_dram_parameter("wk", [128, dim_t * HDL], BF, isOutput=False)
    wv_d = nc.declare_dram_parameter("wv", [128, dim_t * HDL], BF, isOutput=False)
    wo_d = nc.declare_dram_parameter("wo", [128, 4 * DIM], BF, isOutput=False)
    bq_d = nc.declare_dram_parameter("bq8", [128, 4], FP32, isOutput=False)
    out_d = nc.declare_dram_parameter("out", [seq, DIM], BF, isOutput=True)
    rrs_d = nc.dram_tensor("rrs", [4, n_qc, 2, 512], FP32)

    env = dict(locals())

    with tile.TileContext(nc, pool_alloc_mode="queue") as tc:
        with tc.tile_pool(name="persist", bufs=1) as persist:
            xt_sb = persist.tile([128, dim_t * seq], BF, name="xt_sb")
            wq_sb = persist.tile([128, dim_t * HDL], BF, name="wq_sb")
            wk_sb = persist.tile([128, dim_t * HDL], BF, name="wk_sb")
            wv_sb = persist.tile([128, dim_t * HDL], BF, name="wv_sb")
            wo_sb = persist.tile([128, 4 * DIM], BF, name="wo_sb")
            bq_sb = persist.tile([128, 4], FP32, name="bq_sb")
            # Q fp8 (value, residual) pairs and K fp8 (dup) per head-pair m
            QP = [persist.tile([128, 2 * seq], F8, name=f"qp{m}") for m in range(4)]
            KD = [persist.tile([128, 2 * seq], F8, name=f"kd{m}") for m in range(4)]
            # V fp8 value/residual, keys-pair-slabbed, 65 cols per head
            VA = [persist.tile([128, 2 * NH_LOC * 80], F8, name=f"va{k}")
                  for k in range(n_ktp)]
            VB = [persist.tile([128, 2 * NH_LOC * 80], F8, name=f"vb{k}")
                  for k in range(n_ktp)]
            # 32*ctx in bf16: [128, o4, 512] per q-chunk
            UT = [persist.tile([128, 4 * 512], BF, name=f"ut{q}")
                  for q in range(n_qc)]

            rep_ctx = tc.For_i(0, reps, 1) if reps > 1 else contextlib.nullcontext()
            with rep_ctx:
                env.update(locals())
                _build_body(nc, tc, bass, mybir, tile, env)

    nc.compile()
    return nc


def _build_body(nc, tc, bass, mybir, tile, env):
    FP32 = mybir.dt.float32
    BF = mybir.dt.bfloat16
    F8 = mybir.dt.float8e4
    I8 = mybir.dt.int8
    Exp = mybir.ActivationFunctionType.Exp
    Ident = mybir.ActivationFunctionType.Identity
    Copy = mybir.ActivationFunctionType.Copy
    Alu = mybir.AluOpType
    DR = mybir.MatmulPerfMode.DoubleRow

    seq, seq_t, n_qc, n_ktp, dim_t = (env[k] for k in
        ["seq", "seq_t", "n_qc", "n_ktp", "dim_t"])
    xt_d, wq_d, wk_d, wv_d, wo_d, bq_d, out_d, rrs_d = (env[k] for k in
        ["xt_d", "wq_d", "wk_d", "wv_d", "wo_d", "bq_d", "out_d", "rrs_d"])
    xt_sb, wq_sb, wk_sb, wv_sb, wo_sb, bq_sb = (env[k] for k in
        ["xt_sb", "wq_sb", "wk_sb", "wv_sb", "wo_sb", "bq_sb"])
    QP, KD, VA, VB, UT = (env[k] for k in ["QP", "KD", "VA", "VB", "UT"])

    # ---------------- Phase A/B: load + QKV projections ----------------
    xtr = xt_sb[:].rearrange("p (d c) -> p d c", d=dim_t)
    xtdr = xt_d[:].rearrange("p (d c) -> p d c", d=dim_t)
    nc.sync.dma_start(out=bq_sb[:], in_=bq_d[:, :])
    nc.sync.dma_start(out=wq_sb[:], in_=wq_d[:, :])
    for sc in range(seq // 512):
        c0 = sc * 512
        nc.sync.dma_start(out=xtr[:, :, c0:c0 + 512], in_=xtdr[:, :, c0:c0 + 512])
    nc.sync.dma_start(out=wk_sb[:], in_=wk_d[:, :])
    nc.sync.dma_start(out=wv_sb[:], in_=wv_d[:, :])
    nc.sync.dma_start(out=wo_sb[:], in_=wo_d[:, :])
    wqr = wq_sb[:].rearrange("p (d c) -> p d c", d=dim_t)
    wkr = wk_sb[:].rearrange("p (d c) -> p d c", d=dim_t)
    wvr = wv_sb[:].rearrange("p (d c) -> p d c", d=dim_t)
    qpr = [QP[m][:].rearrange("p (s c) -> p s c", s=2) for m in range(4)]
    kdr = [KD[m][:].rearrange("p (s c) -> p s c", s=2) for m in range(4)]
    var = [VA[k][:].rearrange("p (s h c) -> p s h c", s=2, h=NH_LOC)
           for k in range(n_ktp)]
    vbr = [VB[k][:].rearrange("p (s h c) -> p s h c", s=2, h=NH_LOC)
           for k in range(n_ktp)]
    utr = [UT[q][:].rearrange("p (o c) -> p o c", o=4) for q in range(n_qc)]

    with tc.tile_pool(name="qkvps", bufs=4, space="PSUM") as qkvps:
        # ones columns of the V tiles (row 64 of U = softmax denominator / 32)
        for k in range(n_ktp):
            nc.vector.memset(var[k][:, :, :, 64:65], 0.03125)
            nc.vector.memset(vbr[k][:, :, :, 64:65], 0.0)

        # Q^T per 512-col seq chunk (sc outer so compute follows the x DMAs)
        for sc in range(seq // 512):
            c0 = sc * 512
            for m in range(4):
                qp = qkvps.tile([128, 512], FP32, tag="qkv")
                for d in range(dim_t):
                    nc.tensor.matmul(
                        qp[:], wqr[:, d, m * 128:(m + 1) * 128],
                        xtr[:, d, c0:c0 + 512],
                        start=(d == 0), stop=(d == dim_t - 1))
                # Q8a = fp8(8q + 8bq); Q8b = fp8((8q + 8bq) - Q8a)
                nc.scalar.activation(
                    qpr[m][:, 0, c0:c0 + 512], qp[:], Ident,
                    bias=bq_sb[:, m:m + 1], scale=1.0)
                nc.vector.scalar_tensor_tensor(
                    qpr[m][:, 1, c0:c0 + 512], qp[:], bq_sb[:, m:m + 1],
                    qpr[m][:, 0, c0:c0 + 512], Alu.add, Alu.subtract)
        # K^T, duplicated across the two DR slabs via a stride-0 read
        for sc in range(seq // 512):
            c0 = sc * 512
            for m in range(4):
                kp = qkvps.tile([128, 512], FP32, tag="qkv")
                for d in range(dim_t):
                    nc.tensor.matmul(
                        kp[:], wkr[:, d, m * 128:(m + 1) * 128],
                        xtr[:, d, c0:c0 + 512],
                        start=(d == 0), stop=(d == dim_t - 1))
                kap = kp[:]
                kdup = bass.AP(tensor=kap.tensor, offset=kap.offset,
                               ap=[kap.ap[0], [0, 2], [1, 512]])
                nc.scalar.activation(kdr[m][:, :, c0:c0 + 512], kdup, Copy)

        # V natural layout [seq, hd] + residual, strided into VA/VB
        for st in range(seq_t):
            vp = qkvps.tile([128, HDL], FP32, tag="qkv")
            for d in range(dim_t):
                nc.tensor.matmul(
                    vp[:], xtr[:, d, st * 128:(st + 1) * 128], wvr[:, d, :],
                    start=(d == 0), stop=(d == dim_t - 1))
            k, s = st // 2, st % 2
            vp3 = vp[:].rearrange("p (h c) -> p h c", c=HID)
            nc.scalar.activation(var[k][:, s, :, 0:HID], vp3, Copy)
            nc.vector.tensor_tensor(
                vbr[k][:, s, :, 0:HID], vp3, var[k][:, s, :, 0:HID],
                Alu.subtract)

    # ---------------- Phase C: attention ----------------
    with (
        tc.tile_pool(name="e2pool", bufs=4) as e2pool,
        tc.tile_pool(name="rpool", bufs=4) as rpool,
        tc.tile_pool(name="rbpool", bufs=4) as rbpool,
        tc.tile_pool(name="sps", bufs=2, space="PSUM") as sps,
        tc.tile_pool(name="ups", bufs=1, space="PSUM") as ups,
    ):
        def unit(sid, m, qc):
            """One (head-pair m, q-chunk qc) attention unit on stream sid."""
            q0 = qc * 512
            us = [ups.tile([65, 512], FP32, tag=f"u{sid}{h}", name=f"u{sid}{h}")
                  for h in range(2)]
            e2s = []
            steps = []
            attns = []
            for kt in range(seq_t):
                def step(kt=kt):
                    s2 = sps.tile([128, 1024], FP32, tag="s2", name="s2")
                    for h in range(2):
                        hb = h * 64
                        nc.tensor.matmul(
                            s2[:, h * 512:(h + 1) * 512],
                            kdr[m][hb:hb + 64, :, kt * 128:(kt + 1) * 128],
                            qpr[m][hb:hb + 64, :, q0:q0 + 512],
                            start=True, stop=True, perf_mode=DR)
                    if kt % 2 == 0:
                        e2 = e2pool.tile([128, 2 * 1024], F8, tag="e2", name="e2")
                        e2s.append(e2)
                    e2 = e2s[kt // 2]
                    b0 = (kt % 2) * 1024
                    sl = e2[:, b0:b0 + 1024]
                    if sid == 0:
                        nc.scalar.activation(sl, s2[:], Exp, scale=EXP_SCALE)
                    else:
                        nc.vector.tensor_scalar(
                            sl.bitcast(mybir.dt.int8), s2[:],
                            SCHRAUD_MUL, SCHRAUD_OFF, Alu.mult, Alu.add)
                steps.append(step)

                if kt % 2 == 1:
                    def attn(ktp=kt // 2):
                        e2 = e2s[ktp]
                        e2r = e2[:].rearrange("p (s c) -> p s c", s=2)
                        for h in range(2):
                            hg = 2 * m + h
                            rhs = e2r[:, :, h * 512:(h + 1) * 512]
                            nc.tensor.matmul(
                                us[h][:], var[ktp][:, :, hg, 0:65], rhs,
                                start=(ktp == 0), stop=False, perf_mode=DR)
                            nc.tensor.matmul(
                                us[h][:], vbr[ktp][:, :, hg, 0:65], rhs,
                                start=False, stop=(ktp == n_ktp - 1),
                                perf_mode=DR)
                    attns.append(attn)

            def finish():
                for h in range(2):
                    hg = 2 * m + h
                    rr = rpool.tile([1, 512], FP32, tag="rr")
                    nc.vector.reciprocal(rr[:], us[h][64:65, :])
                    slot = rrs_d[m, qc, h, :]
                    nc.gpsimd.dma_start(out=slot, in_=rr[0:1, :])
                    rb = rbpool.tile([64, 512], FP32, tag="rb")
                    nc.gpsimd.dma_start(
                        out=rb[:],
                        in_=bass.AP(tensor=slot.tensor, offset=slot.offset,
                                    ap=[[0, 64], [1, 512]]))
                    # UT = U[0:64] * (32/r)  (= 32*ctx, bf16)
                    nc.vector.tensor_tensor(
                        utr[qc][(hg % 2) * 64:(hg % 2) * 64 + 64, hg // 2, :],
                        us[h][0:HID, :], rb[:], Alu.mult)
            return steps, attns, finish

        for qc in range(n_qc):
            for mp in range(2):
                pair = [unit(0, 2 * mp, qc), unit(1, 2 * mp + 1, qc)]
                for kt in range(seq_t):
                    # scores for both streams first, then dependent work
                    for st_, at_, fi_ in pair:
                        st_[kt]()
                    if kt % 2 == 1 and kt >= 3:
                        for st_, at_, fi_ in pair:
                            at_[kt // 2 - 1]()
                for st_, at_, fi_ in pair:
                    at_[n_ktp - 1]()
                for st_, at_, fi_ in pair:
                    fi_()

    # ---------------- Phase D: out projection (bf16) ----------------
    wor = wo_sb[:].rearrange("p (o c) -> p o c", o=4)
    with (
        tc.tile_pool(name="outstage", bufs=3) as outstage,
        tc.tile_pool(name="ops", bufs=4, space="PSUM") as ops,
    ):
        for st in range(seq_t):
            qc, c0 = st // 4, (st % 4) * 128
            ot = outstage.tile([128, DIM], BF, tag="ot")
            for oc in range(2):
                op_t = ops.tile([128, 512], FP32, tag="op")
                for o in range(4):
                    nc.tensor.matmul(
                        op_t[:], utr[qc][:, o, c0:c0 + 128],
                        wor[:, o, oc * 512:(oc + 1) * 512],
                        start=(o == 0), stop=(o == 3))
                # UT holds 32*ctx -> scale back here
                if oc == 0:
                    nc.scalar.activation(ot[:, oc * 512:(oc + 1) * 512],
                                         op_t[:], Copy, scale=0.03125)
                else:
                    nc.vector.tensor_scalar(ot[:, oc * 512:(oc + 1) * 512],
                                            op_t[:], 0.03125, None, Alu.mult)
            nc.gpsimd.dma_start(out=out_d[st * 128:(st + 1) * 128, :], in_=ot[:])


def _get_program():
    global _PROG
    if _PROG is None:
        _PROG = _build_program()
    return _PROG


def _prep_core_inputs(x, Wq, bq, Wk, Wv, Wo):
    """Host-side layout/cast for one core: x [2048, 1024] f32, W* pre-sliced."""
    xt = np.ascontiguousarray(x.T).astype(BF16)            # [1024, 2048]
    xt = xt.reshape(8, 128, SEQ).transpose(1, 0, 2).reshape(128, 8 * SEQ)
    def wlay(w):                                           # [1024, 512]
        w = np.asarray(w, np.float32).astype(BF16)
        return w.reshape(8, 128, HDL).transpose(1, 0, 2).reshape(128, 8 * HDL)
    wo = np.asarray(Wo, np.float32).astype(BF16)           # [512, 1024]
    wo = wo.reshape(4, 128, DIM).transpose(1, 0, 2).reshape(128, 4 * DIM)
    bq8 = np.ascontiguousarray(
        (8.0 * np.asarray(bq, np.float32)).reshape(4, 128).T)
    return {
        "xt": np.ascontiguousarray(xt),
        "wq": np.ascontiguousarray(wlay(8.0 * np.asarray(Wq, np.float32))),
        "wk": np.ascontiguousarray(wlay(Wk)),
        "wv": np.ascontiguousarray(wlay(Wv)),
        "wo": np.ascontiguousarray(wo),
        "bq8": bq8,
    }


def _make_in_maps(inputs):
    x = np.asarray(inputs["x"], dtype=np.float32)
    Wq = np.asarray(inputs["Wq"], np.float32)
    Wk = np.asarray(inputs["Wk"], np.float32)
    Wv = np.asarray(inputs["Wv"], np.float32)
    Wo = np.asarray(inputs["Wo"], np.float32)
    bq = np.asarray(inputs["bq"], np.float32)
    in_maps = []
    for c in range(N_CORES):
        b, g = divmod(c, 2)
        sl = slice(g * HDL, (g + 1) * HDL)
        in_maps.append(_prep_core_inputs(
            x[b], Wq[:, sl], bq[sl], Wk[:, sl], Wv[:, sl], Wo[sl, :]))
    return in_maps


def kernel(x, Wq, bq, Wk, bk, Wv, bv, Wo, bo):
    from concourse.bass_utils import run_bass_kernel_spmd

    bo = np.asarray(bo, dtype=np.float32)
    bv = np.asarray(bv, dtype=np.float32)
    Wo_f = np.asarray(Wo, np.float32)
    nc = _get_program()
    in_maps = _make_in_maps(dict(x=x, Wq=Wq, bq=bq, Wk=Wk, Wv=Wv, Wo=Wo))
    res = run_bass_kernel_spmd(nc, in_maps, core_ids=list(range(N_CORES)))
    extra = bo + bv @ Wo_f           # bv folds through the attention average
    out = np.empty((B, SEQ, DIM), dtype=np.float32)
    for b in range(B):
        out[b] = (res.results[2 * b]["out"].astype(np.float32)
                  + res.results[2 * b + 1]["out"].astype(np.float32) + extra)
    return out
